# revision 7
# baseline (speedup 1.0000x reference)
"""4-layer GraphSAGE (mean aggr) on 8 TRN2 NeuronCores.

Strategy (graph/data parallel, dst-owner node partitioning):
  - Nodes are partitioned across the 8 cores by dst ownership (12500 each,
    padded to 12544 = 98*128).  Each core aggregates the in-edges of its own
    nodes: per-edge dma_gather of src features from a replicated node-feature
    table in DRAM, then dma_scatter_add into a per-core accumulator.
  - The gather index is int16, so the 100352-row table is addressed in 4
    ranges of 25088 rows; edges are grouped per (core, src-range) host-side.
  - Layer 1 is "transform-first": table1 = x @ Wl1 (so every layer gathers
    128-wide rows), self term uses x^T directly.
  - After aggregation: normalize by 1/deg (ACT per-partition scale),
    PE-transpose to feature-major, weight-stationary matmuls
    (agg @ Wl + h @ Wr + b, ReLU between layers), transpose back to
    node-major, AllGather shards into the next layer's table.  h^T is
    kept feature-major in DRAM and streamed per 512-node chunk.
"""

import numpy as np

# ---------------------------------------------------------------- constants
NCORES = 8
N = 100000
E = 1600000
F_IN = 16
H = 128
SHARD = 12500            # real nodes owned per core
BLK = 128
NBLK = 98                # 98*128 = 12544
SHARD_P = NBLK * BLK     # padded shard rows
TBL_ROWS = NCORES * SHARD_P   # 100352
NRANGE = 4
RANGE_ROWS = TBL_ROWS // NRANGE  # 25088 (< 2**15)
CHUNK = 1024             # edges per gather/scatter instruction (HW limit)
NCHUNK_R = 50            # chunks per (core, src-range)
CAP_R = NCHUNK_R * CHUNK  # 51200 edge capacity per (core, src-range)
JUNK_ROW = SHARD_P - 1   # scatter target for padding edges (a pad node)

_compiled = None


# ---------------------------------------------------------------- program
def _build_program():
    import concourse.bacc as bacc
    import concourse.masks as masks
    import concourse.mybir as mybir
    import concourse.tile as tile

    fp32 = mybir.dt.float32
    i16 = mybir.dt.int16
    AF = mybir.ActivationFunctionType

    nc = bacc.Bacc(
        "TRN2",
        target_bir_lowering=False,
        debug=False,
        enable_asserts=False,
        num_devices=NCORES,
    )

    # -------- I/O declarations
    xt_d = nc.dram_tensor("xt", [F_IN, SHARD_P], fp32, kind="ExternalInput")
    gidx_d = nc.dram_tensor(
        "gidx", [128, NRANGE * CAP_R // 16], i16, kind="ExternalInput"
    )
    sidx_d = nc.dram_tensor(
        "sidx", [128, NRANGE * CAP_R // 16], i16, kind="ExternalInput"
    )
    invc_d = nc.dram_tensor("invc", [128, NBLK], fp32, kind="ExternalInput")
    w_d = {}
    for l in range(1, 5):
        din = F_IN if l == 1 else H
        w_d[f"wl{l}"] = nc.dram_tensor(f"wl{l}", [din, H], fp32, kind="ExternalInput")
        w_d[f"wr{l}"] = nc.dram_tensor(f"wr{l}", [din, H], fp32, kind="ExternalInput")
        w_d[f"b{l}"] = nc.dram_tensor(f"b{l}", [128, 1], fp32, kind="ExternalInput")

    out_d = nc.dram_tensor("out", [SHARD_P, H], fp32, kind="ExternalOutput")

    with tile.TileContext(nc) as tc:
        with (
            tc.tile_pool(name="dram", bufs=1, space="DRAM") as dpool,
            tc.tile_pool(name="const", bufs=1) as cpool,
            tc.tile_pool(name="gat", bufs=6) as gpool,
            tc.tile_pool(name="work", bufs=3) as wpool,
            tc.tile_pool(name="psum_o", bufs=2, space="PSUM") as popool,
            tc.tile_pool(name="psum_t", bufs=4, space="PSUM") as ptpool,
        ):
            sh = [dpool.tile([SHARD_P, H], fp32, name=f"sh{l}") for l in range(4)]
            acc_d = [
                dpool.tile([SHARD_P, H], fp32, name=f"acc{l}") for l in range(1, 5)
            ]
            tbl = [
                dpool.tile([TBL_ROWS, H], fp32, addr_space="Shared", name=f"tbl{l}")
                for l in range(1, 5)
            ]
            # feature-major h^T, streamed per chunk
            ht_dram = [
                dpool.tile([128, SHARD_P], fp32, name=f"ht{l}") for l in range(1, 4)
            ]

            # -------- constants to SBUF
            ident = cpool.tile([128, 128], fp32)
            masks.make_identity(nc, ident[:])
            xt_sb = cpool.tile([F_IN, SHARD_P], fp32)
            nc.sync.dma_start(xt_sb[:], xt_d.ap())
            invc_sb = cpool.tile([128, NBLK], fp32)
            nc.sync.dma_start(invc_sb[:], invc_d.ap())
            gidx_sb = cpool.tile([128, NRANGE * CAP_R // 16], i16)
            nc.sync.dma_start(gidx_sb[:], gidx_d.ap())
            sidx_sb = cpool.tile([128, NRANGE * CAP_R // 16], i16)
            nc.sync.dma_start(sidx_sb[:], sidx_d.ap())
            w_sb = {}
            for l in range(1, 5):
                din = F_IN if l == 1 else H
                for nm in (f"wl{l}", f"wr{l}"):
                    t = cpool.tile([din, H], fp32, name=f"{nm}_sb")
                    nc.sync.dma_start(t[:], w_d[nm].ap())
                    w_sb[nm] = t
                t = cpool.tile([128, 1], fp32, name=f"b{l}_sb")
                nc.sync.dma_start(t[:], w_d[f"b{l}"].ap())
                w_sb[f"b{l}"] = t

            # node-major view of DRAM row blocks: row n = b*128 + p
            def nm_view(dram_ap):
                return dram_ap.rearrange("(b p) f -> p b f", p=128)

            groups = [(i, min(4, NBLK - i)) for i in range(0, NBLK, 4)]

            def emit_nm(src_sb, cw, dst_view, b0, nb, tag):
                """src_sb [128f, cw] feature-major chunk -> node-major DRAM
                rows (blocks b0..b0+nb) via PE transposes."""
                stage = wpool.tile([128, 4, 128], fp32, tag=f"nm_{tag}")
                for j in range(nb):
                    pt = ptpool.tile([128, 128], fp32, tag="pt")
                    nc.tensor.transpose(
                        pt[:], src_sb[:, j * 128 : (j + 1) * 128], ident[:]
                    )
                    nc.vector.tensor_copy(stage[:, j, :], pt[:])
                nc.sync.dma_start(dst_view[:, b0 : b0 + nb, :], stage[:, :nb, :])

            def allgather(src, dst):
                nc.gpsimd.collective_compute(
                    "AllGather",
                    mybir.AluOpType.bypass,
                    replica_groups=[list(range(NCORES))],
                    ins=[src.opt()],
                    outs=[dst.opt()],
                )

            # zero the scatter accumulators on device
            zt = cpool.tile([128, 4, 128], fp32, name="zt")
            nc.gpsimd.memset(zt[:], 0.0)
            for a in acc_d:
                av = nm_view(a)
                for b0, nb in groups:
                    nc.sync.dma_start(av[:, b0 : b0 + nb, :], zt[:, :nb, :])

            # ---- layer 1 transform: table1 = x @ Wl1 (own shard, allgather)
            sh0v = nm_view(sh[0])
            for b0, nb in groups:
                cw = nb * 128
                sl = slice(b0 * 128, b0 * 128 + cw)
                ps = popool.tile([128, 512], fp32, tag="ps")
                nc.tensor.matmul(
                    ps[:, :cw], w_sb["wl1"][:], xt_sb[:, sl], start=True, stop=True
                )
                tmp = wpool.tile([128, 512], fp32, tag="x1tmp")
                nc.scalar.copy(tmp[:, :cw], ps[:, :cw])
                emit_nm(tmp, cw, sh0v, b0, nb, "x1")
            allgather(sh[0], tbl[0])

            # ---- layers
            for l in range(1, 5):
                table = tbl[l - 1]
                acc = acc_d[l - 1]

                # phase B: gather + scatter-add over all edges.  Each chunk
                # has at most one edge per dst (host-side dealing), so the
                # scatter-add RMW never collides within an instruction;
                # consecutive scatters on the same acc are serialized by Tile.
                for g in range(NRANGE):
                    tslice = table[g * RANGE_ROWS : (g + 1) * RANGE_ROWS, :]
                    for ci in range(NCHUNK_R):
                        c0 = (g * CAP_R + ci * CHUNK) // 16
                        c1 = c0 + CHUNK // 16
                        gt = gpool.tile([128, CHUNK // 128, H], fp32, tag="gt")
                        nc.gpsimd.dma_gather(
                            gt[:],
                            tslice,
                            gidx_sb[:, c0:c1],
                            num_idxs=CHUNK,
                            num_idxs_reg=CHUNK,
                            elem_size=H,
                        )
                        nc.gpsimd.dma_scatter_add(
                            acc[:, :],
                            gt[:],
                            sidx_sb[:, c0:c1],
                            num_idxs=CHUNK,
                            num_idxs_reg=CHUNK,
                            elem_size=H,
                        )

                # phase C: readback, normalize, transpose, matmul, evict
                accv = nm_view(acc)
                dst_view = nm_view(sh[l] if l < 4 else out_d.ap())
                for b0, nb in groups:
                    cw = nb * 128
                    sl = slice(b0 * 128, b0 * 128 + cw)
                    agg_nm = wpool.tile([128, 4, 128], fp32, tag="agg_nm")
                    nc.sync.dma_start(agg_nm[:, :nb, :], accv[:, b0 : b0 + nb, :])
                    aggT = wpool.tile([128, 512], fp32, tag="aggT")
                    for j in range(nb):
                        b = b0 + j
                        nt = wpool.tile([128, 128], fp32, tag="norm")
                        nc.scalar.mul(nt[:], agg_nm[:, j, :], invc_sb[:, b : b + 1])
                        pt = ptpool.tile([128, 128], fp32, tag="pt")
                        nc.tensor.transpose(pt[:], nt[:], ident[:])
                        nc.vector.tensor_copy(aggT[:, j * 128 : (j + 1) * 128], pt[:])
                    ps = popool.tile([128, 512], fp32, tag="ps")
                    if l == 1:
                        nc.tensor.matmul(
                            ps[:, :cw], w_sb["wr1"][:], xt_sb[:, sl],
                            start=True, stop=False,
                        )
                        nc.tensor.matmul(
                            ps[:, :cw], ident[:], aggT[:, :cw],
                            start=False, stop=True,
                        )
                    else:
                        hc = wpool.tile([128, 512], fp32, tag="hc")
                        nc.sync.dma_start(hc[:, :cw], ht_dram[l - 2][:, sl])
                        nc.tensor.matmul(
                            ps[:, :cw], w_sb[f"wl{l}"][:], aggT[:, :cw],
                            start=True, stop=False,
                        )
                        nc.tensor.matmul(
                            ps[:, :cw], w_sb[f"wr{l}"][:], hc[:, :cw],
                            start=False, stop=True,
                        )
                    func = AF.Relu if l < 4 else AF.Identity
                    ev = wpool.tile([128, 512], fp32, tag="ev")
                    nc.scalar.activation(
                        ev[:, :cw], ps[:, :cw], func, bias=w_sb[f"b{l}"][:]
                    )
                    if l < 4:
                        nc.sync.dma_start(ht_dram[l - 1][:, sl], ev[:, :cw])
                    emit_nm(ev, cw, dst_view, b0, nb, "h")

                if l < 4:
                    allgather(sh[l], tbl[l])

    nc.compile()
    return nc


def _get_program():
    global _compiled
    if _compiled is None:
        _compiled = _build_program()
    return _compiled


# ---------------------------------------------------------------- host side
def _wrap_idx(a):
    """[L] int16 -> [128, L/16] layout: idx j at [j%16, j//16], replicated
    across the 8 groups of 16 partitions."""
    a2 = a.reshape(-1, 16).T.copy()
    return np.tile(a2, (8, 1))


def make_in_maps(x, edge_index, weights):
    src = np.asarray(edge_index[0], dtype=np.int64)
    dst = np.asarray(edge_index[1], dtype=np.int64)
    x = np.asarray(x, dtype=np.float32)

    cnt = np.bincount(dst, minlength=N).astype(np.float32)
    inv_full = (1.0 / np.maximum(cnt, 1.0)).astype(np.float32)

    core = dst // SHARD
    dst_loc = (dst - core * SHARD).astype(np.int64)
    src_row = (src // SHARD) * SHARD_P + (src % SHARD)
    rng = src_row // RANGE_ROWS
    src_loc = (src_row - rng * RANGE_ROWS).astype(np.int64)

    in_maps = []
    for c in range(NCORES):
        m = core == c
        gi = np.zeros(NRANGE * CAP_R, np.int16)
        si = np.full(NRANGE * CAP_R, JUNK_ROW, np.int16)
        for g in range(NRANGE):
            sel = m & (rng == g)
            k = int(sel.sum())
            assert k <= CAP_R, f"core {c} range {g}: {k} > {CAP_R}"
            s_g = src_loc[sel]
            d_g = dst_loc[sel]
            # group edges by dst, then deal to chunks round-robin: sorted
            # position i -> chunk i % NCHUNK_R.  Same-dst edges (consecutive
            # after the sort, degree <= NCHUNK_R) land in distinct chunks and
            # chunk loads are balanced to +-1.
            order = np.argsort(d_g, kind="stable")
            s_g, d_g = s_g[order], d_g[order]
            deg_max = np.bincount(d_g).max() if k else 0
            assert deg_max <= NCHUNK_R, f"deg {deg_max} > {NCHUNK_R}"
            chunk = np.arange(k) % NCHUNK_R
            # within each chunk, sort by src for gather locality
            order2 = np.lexsort((s_g, chunk))
            s_g, d_g, chunk = s_g[order2], d_g[order2], chunk[order2]
            loads = np.bincount(chunk, minlength=NCHUNK_R)
            starts = np.concatenate([[0], np.cumsum(loads)[:-1]])
            within = np.arange(k) - starts[chunk]
            slot = g * CAP_R + chunk * CHUNK + within
            gi[slot] = s_g.astype(np.int16)
            si[slot] = d_g.astype(np.int16)

        xt = np.zeros((F_IN, SHARD_P), np.float32)
        xt[:, :SHARD] = x[c * SHARD : (c + 1) * SHARD].T

        invc = np.zeros(SHARD_P, np.float32)
        invc[:SHARD] = inv_full[c * SHARD : (c + 1) * SHARD]
        invc = invc.reshape(NBLK, 128).T.copy()

        im = {
            "xt": xt,
            "gidx": _wrap_idx(gi),
            "sidx": _wrap_idx(si),
            "invc": invc,
        }
        for l in range(1, 5):
            im[f"wl{l}"] = np.asarray(weights[f"Wl{l}"], np.float32)
            im[f"wr{l}"] = np.asarray(weights[f"Wr{l}"], np.float32)
            im[f"b{l}"] = np.asarray(weights[f"b{l}"], np.float32).reshape(128, 1)
        in_maps.append(im)
    return in_maps


def bench_exec(nc, in_maps, iters=5):
    """Mirror of bass2jax.run_bass_via_pjrt's multi-core path, but jits once,
    keeps inputs on device, and times repeated executions."""
    import time

    import jax
    import numpy as np_
    from jax.sharding import Mesh, PartitionSpec
    from jax.experimental.shard_map import shard_map

    from concourse import bass2jax, mybir

    bass2jax.install_neuronx_cc_hook()
    partition_name = (
        nc.partition_id_tensor.name if nc.partition_id_tensor else None
    )
    in_names, out_names, out_avals = [], [], []
    for alloc in nc.m.functions[0].allocations:
        if not isinstance(alloc, mybir.MemoryLocationSet):
            continue
        name = alloc.memorylocations[0].name
        if alloc.kind == "ExternalInput":
            if name != partition_name:
                in_names.append(name)
        elif alloc.kind == "ExternalOutput":
            out_names.append(name)
            out_avals.append(
                jax.core.ShapedArray(
                    tuple(alloc.tensor_shape), mybir.dt.np(alloc.dtype)
                )
            )
    n_params = len(in_names)
    all_in_names = list(in_names)
    if partition_name is not None:
        all_in_names.append(partition_name)

    def _body(*args):
        operands = list(args)
        if partition_name is not None:
            operands.append(bass2jax.partition_id_tensor())
        return tuple(
            bass2jax._bass_exec_p.bind(
                *operands,
                out_avals=tuple(out_avals),
                in_names=tuple(all_in_names),
                out_names=tuple(out_names),
                lowering_input_output_aliases=(),
                sim_require_finite=True,
                sim_require_nnan=True,
                nc=nc,
            )
        )

    n_cores = len(in_maps)
    devices = jax.devices()[:n_cores]
    mesh = Mesh(np_.asarray(devices), ("core",))
    fn = jax.jit(
        shard_map(
            _body,
            mesh=mesh,
            in_specs=(PartitionSpec("core"),) * n_params,
            out_specs=(PartitionSpec("core"),) * len(out_names),
            check_rep=False,
        ),
        keep_unused=True,
    )
    concat_in = [
        np_.concatenate([np_.asarray(in_maps[c][nm]) for c in range(n_cores)], axis=0)
        for nm in in_names
    ]
    dev_in = [jax.device_put(a) for a in concat_in]
    outs = fn(*dev_in)
    jax.block_until_ready(outs)
    times = []
    for _ in range(iters):
        t0 = time.perf_counter()
        outs = fn(*dev_in)
        jax.block_until_ready(outs)
        times.append(time.perf_counter() - t0)
    results = [
        {nm: np_.asarray(outs[i]).reshape(n_cores, *out_avals[i].shape)[c]
         for i, nm in enumerate(out_names)}
        for c in range(n_cores)
    ]
    return results, times


def kernel(x, edge_index, Wl1, Wr1, b1, Wl2, Wr2, b2, Wl3, Wr3, b3,
           Wl4, Wr4, b4, _trace=False, _trace_kwargs=None):
    from concourse.bass_utils import run_bass_kernel_spmd

    weights = {
        "Wl1": Wl1, "Wr1": Wr1, "b1": b1,
        "Wl2": Wl2, "Wr2": Wr2, "b2": b2,
        "Wl3": Wl3, "Wr3": Wr3, "b3": b3,
        "Wl4": Wl4, "Wr4": Wr4, "b4": b4,
    }
    nc = _get_program()
    in_maps = make_in_maps(x, edge_index, weights)
    res = run_bass_kernel_spmd(
        nc,
        in_maps,
        core_ids=list(range(NCORES)),
        trace=_trace,
        **(_trace_kwargs or {}),
    )
    shards = [res.results[c]["out"][:SHARD] for c in range(NCORES)]
    out = np.concatenate(shards, axis=0).astype(np.float32)
    if _trace:
        return out, res
    return out
